# revision 1
# baseline (speedup 1.0000x reference)
"""Binary-weight dense layer on 8 trn2 NeuronCores.

Computes out[b,s,f] = scale * sum_i x[b,s,i] * (kernel[i,f] ? +1 : -1)
for x [4, 4096, 1024] f32, kernel [1024, 1024] bool, scale scalar f32.

Strategy: data-parallel over the 16384 rows (2048 rows/core).  Host-side
prep transposes each x shard to [K, rows] bf16 so SBUF tiles land in
matmul layout with fully-contiguous DMA lines, and folds scale into the
+-1 weights (exact in bf16 for power-of-two scales).  On-chip: pure
bf16 matmul accumulating fp32 in PSUM, DVE copy to SBUF, DMA out.
"""

import numpy as np
import ml_dtypes

import concourse.bacc as bacc
import concourse.mybir as mybir
import concourse.tile as tile
from concourse.bass_utils import run_bass_kernel_spmd

N_CORES = 8
B, S, K, N = 4, 4096, 1024, 1024
ROWS = B * S                    # 16384
ROWS_PER_CORE = ROWS // N_CORES  # 2048
P = 128                         # partitions
KT = K // P                     # 8 contraction subtiles
MT = ROWS_PER_CORE // P         # 16 row tiles per core
NHALF = 512                     # one PSUM bank of f32

_module_cache = {}


def build_module():
    nc = bacc.Bacc(None)
    xt = nc.dram_tensor("xt", [K, ROWS_PER_CORE], mybir.dt.bfloat16,
                        kind="ExternalInput")
    w = nc.dram_tensor("w", [K, N], mybir.dt.bfloat16, kind="ExternalInput")
    out = nc.dram_tensor("out", [ROWS_PER_CORE, N], mybir.dt.float32,
                         kind="ExternalOutput")

    HROWS = ROWS_PER_CORE // 2   # 1024 rows per x half-chunk
    G0 = 4                       # m-tiles processed k-major during load phase

    with tile.TileContext(nc) as tc:
        with (
            tc.tile_pool(name="persist", bufs=1) as persist,
            tc.tile_pool(name="psum", bufs=1, space="PSUM") as ps_pool,
            tc.tile_pool(name="outp", bufs=3) as out_pool,
        ):
            # Dummy matmuls fill the PE-idle window while the first input
            # chunks are in flight, so the HAM clock-gate is already
            # released (2.4 GHz) when the real stream starts.  The buffer
            # is memset on GpSimd (runs right after Tile's own const
            # memsets) -- reading uninitialized SBUF faults the device.
            wu = persist.tile([P, 384], mybir.dt.bfloat16, tag="wu")
            nc.gpsimd.memset(wu, 0)
            warm_ps = ps_pool.tile([P, N], mybir.dt.float32, tag="ps0",
                                   name="warmps")
            for _ in range(16):
                nc.tensor.matmul(warm_ps[:, 0:256], wu[:, 0:P],
                                 wu[:, P:384], start=True, stop=True)

            # Inputs alternate across the two HWDGE rings (SP + ACT
            # sequencers issue in parallel; per-queue throughput is only
            # ~200-260 GB/s, two queues reach fabric rate).  The first
            # matmul's pieces (x0 m-group piece, w0 half) are first on
            # each ring, so they complete ~1 us after the preamble.
            GROWS = G0 * P  # 512 rows per phase-1 piece

            # Ring split with strict FIFO need-ordering per ring (priority
            # across queues does not exist -- concurrent queues just split
            # bandwidth): Sync carries x-g0 + w_b in consumption order;
            # Scalar carries w_a, then the phase-2 x bulk, then all
            # outputs.
            w_half = [[None] * 2 for _ in range(KT)]
            x_g0 = [None] * KT
            x_g1 = [None] * KT
            x_h1 = [None] * KT
            for k in range(KT):
                xc = persist.tile([P, GROWS], mybir.dt.bfloat16, tag=f"x{k}g0")
                nc.sync.dma_start(out=xc, in_=xt[k * P:(k + 1) * P, 0:GROWS])
                x_g0[k] = xc
                for j in range(2):
                    wt = persist.tile([P, NHALF], mybir.dt.bfloat16,
                                      tag=f"w{k}j{j}", name=f"w{k}j{j}")
                    ring = nc.scalar if j == 0 else nc.sync
                    ring.dma_start(out=wt,
                                   in_=w[k * P:(k + 1) * P,
                                         j * NHALF:(j + 1) * NHALF])
                    w_half[k][j] = wt
            for k in range(KT):
                xc = persist.tile([P, GROWS], mybir.dt.bfloat16, tag=f"x{k}g1")
                nc.scalar.dma_start(out=xc,
                                    in_=xt[k * P:(k + 1) * P, GROWS:2 * GROWS])
                x_g1[k] = xc
            for k in range(KT):
                xc = persist.tile([P, HROWS], mybir.dt.bfloat16, tag=f"x{k}h1")
                nc.scalar.dma_start(out=xc,
                                    in_=xt[k * P:(k + 1) * P,
                                           HROWS:ROWS_PER_CORE])
                x_h1[k] = xc

            ps_tiles = {}

            def mm(m, k):
                g, off = divmod(m, G0)
                if g == 0:
                    lhsT = x_g0[k][:, off * P:(off + 1) * P]
                elif g == 1:
                    lhsT = x_g1[k][:, off * P:(off + 1) * P]
                else:
                    o = m * P - HROWS
                    lhsT = x_h1[k][:, o:o + P]
                ps = ps_tiles[m % G0]
                nc.tensor.matmul(ps[:, 0:NHALF], lhsT, w_half[k][0],
                                 start=(k == 0), stop=(k == KT - 1))
                nc.tensor.matmul(ps[:, NHALF:N], lhsT, w_half[k][1],
                                 start=(k == 0), stop=(k == KT - 1))

            def evict(m):
                ot = out_pool.tile([P, N], mybir.dt.float32, tag="ot")
                if m == MT - 1:
                    # last tile: half copies + stores on both rings so the
                    # first half's transfer overlaps the second's copy
                    nc.vector.tensor_copy(ot[:, 0:NHALF],
                                          ps_tiles[m % G0][:, 0:NHALF])
                    nc.sync.dma_start(out=out[m * P:(m + 1) * P, 0:NHALF],
                                      in_=ot[:, 0:NHALF])
                    nc.vector.tensor_copy(ot[:, NHALF:N],
                                          ps_tiles[m % G0][:, NHALF:N])
                    nc.scalar.dma_start(out=out[m * P:(m + 1) * P, NHALF:N],
                                        in_=ot[:, NHALF:N])
                else:
                    nc.vector.tensor_copy(ot, ps_tiles[m % G0])
                    nc.scalar.dma_start(out=out[m * P:(m + 1) * P, :], in_=ot)

            # Phase 1: first G0 m-tiles k-major, consuming chunks as they
            # arrive from DMA.
            for m in range(G0):
                ps_tiles[m] = ps_pool.tile([P, N], mybir.dt.float32,
                                           tag=f"ps{m}", name=f"ps{m}")
            for k in range(KT):
                for m in range(G0):
                    mm(m, k)
            for m in range(G0):
                evict(m)

            # Phase 2: remaining m-tiles m-major (inputs now resident),
            # copy-out pipelined with the next tile's matmuls.
            for m in range(G0, MT):
                ps_tiles[m % G0] = ps_pool.tile([P, N], mybir.dt.float32,
                                                tag=f"ps{m % G0}",
                                                name=f"ps{m}")
                for k in range(KT):
                    mm(m, k)
                evict(m)
    nc.finalize()
    return nc


def get_module():
    if "nc" not in _module_cache:
        _module_cache["nc"] = build_module()
    return _module_cache["nc"]


def _prepare_in_maps(x, kernel, scale):
    bf16 = ml_dtypes.bfloat16
    x2d = np.asarray(x, dtype=np.float32).reshape(ROWS, K)
    scale = np.float32(scale)
    w_signed = np.where(np.asarray(kernel, dtype=bool), scale, -scale)
    w_bf16 = np.ascontiguousarray(w_signed.astype(bf16))
    in_maps = []
    for c in range(N_CORES):
        shard = x2d[c * ROWS_PER_CORE:(c + 1) * ROWS_PER_CORE]
        xt_c = np.ascontiguousarray(shard.T.astype(bf16))
        in_maps.append({"xt": xt_c, "w": w_bf16})
    return in_maps


def kernel(x, kernel, scale):
    nc = get_module()
    in_maps = _prepare_in_maps(x, kernel, scale)
    res = run_bass_kernel_spmd(nc, in_maps, core_ids=list(range(N_CORES)))
    out = np.concatenate([r["out"] for r in res.results], axis=0)
    return out.reshape(B, S, N)

